# revision 7
# baseline (speedup 1.0000x reference)
"""DeepSeek-V2 MLA attention on 8 Trainium2 NeuronCores (Bass/Tile).

Strategy:
  Phase 1 (sequence-sharded): each core computes its 256-row slice of the
    fused down-projection  hidden @ [Wq_down | Wkv_down].T.
  Host glue: bias add, RMSNorm, k-RoPE, transposes (O(S*D) only).
  Phase 2 (head-sharded, 2 heads/core): q/kv up-projections, on-device
    q-RoPE (rotate-half folded into an extra weight block), causal
    attention with scores held transposed [keys, queries], softmax
    denominator via ones-matmul, ctx, and a fused partial out-projection
    interleaved per query tile.
  Host gather: sum the 8 partial out projections, transpose attn.

Matmul dtype: bf16 (fp32 PSUM accumulation) by default; MLA_DTYPE=f32r
switches to full-rate fp32r.
"""
import contextlib
import ctypes
import glob as _glob
import os
import sys

import numpy as np

# The grading environment may pin JAX_PLATFORMS=cpu (needed for the jax
# reference); the axon trn2 devices must stay visible for this kernel.
if os.environ.get("JAX_PLATFORMS", "").lower() in ("cpu",):
    os.environ["JAX_PLATFORMS"] = ""

B, S, HID = 1, 2048, 2048
H = 16
NOPE, ROPE, VD = 128, 64, 128
QD = NOPE + ROPE
QLR, KVLR = 1536, 512
THETA = 10000.0
EPS = 1e-6

NCORES = 8
HL = H // NCORES            # heads per core
SC = S // NCORES            # phase-1 rows per core
DOWN = QLR + KVLR + ROPE    # 2112
SQ = 512                    # query-tile (matmul free dim)
NSI = S // SQ               # 4
NKB = S // 128              # 16 key blocks
SCALE = 1.0 / float(np.sqrt(QD))

MM_DTYPE = os.environ.get("MLA_DTYPE", "bf16")   # "bf16" | "f32r"

_built = {}


def _np_mm_dtype():
    if MM_DTYPE == "bf16":
        import ml_dtypes
        return ml_dtypes.bfloat16
    return np.float32


def _bass_mods():
    import concourse.bass as bass
    import concourse.mybir as mybir
    import concourse.tile as tile
    from concourse import bacc, bass_utils
    return bass, mybir, tile, bacc, bass_utils


def _build_phase1():
    bass, mybir, tile, bacc, _ = _bass_mods()
    f32 = mybir.dt.float32
    mdt = mybir.dt.bfloat16 if MM_DTYPE == "bf16" else mybir.dt.float32r
    nc = bacc.Bacc("TRN2", target_bir_lowering=False, debug=False,
                   num_devices=NCORES)
    hT = nc.dram_tensor("hT", (HID, SC), mdt, kind="ExternalInput").ap()
    wdT = nc.dram_tensor("wdT", (HID, DOWN), mdt, kind="ExternalInput").ap()
    down = nc.dram_tensor("down", (SC, DOWN), f32, kind="ExternalOutput").ap()

    NK = HID // 128
    ntiles = [(0, 512), (512, 512), (1024, 512), (1536, 512), (2048, 64)]
    with tile.TileContext(nc) as tc:
        with (
            tc.tile_pool(name="sb", bufs=1) as sb,
            tc.tile_pool(name="out", bufs=4) as ob,
            tc.tile_pool(name="ps", bufs=2, space="PSUM") as ps,
        ):
            hT_t = sb.tile([128, NK * SC], mdt, name="hT_t")
            wd_t = sb.tile([128, NK * DOWN], mdt, name="wd_t")
            for k in range(NK):
                nc.sync.dma_start(hT_t[:, k * SC:(k + 1) * SC],
                                  hT[k * 128:(k + 1) * 128, :])
                nc.sync.dma_start(wd_t[:, k * DOWN:(k + 1) * DOWN],
                                  wdT[k * 128:(k + 1) * 128, :])
            for st in range(SC // 128):
                for (n0, nw) in ntiles:
                    psm = ps.tile([128, 512], f32, name="psm", tag="p")
                    for k in range(NK):
                        nc.tensor.matmul(
                            psm[:, :nw],
                            hT_t[:, k * SC + st * 128: k * SC + (st + 1) * 128],
                            wd_t[:, k * DOWN + n0: k * DOWN + n0 + nw],
                            start=(k == 0), stop=(k == NK - 1))
                    o = ob.tile([128, 512], f32, name="o", tag="o")
                    nc.any.tensor_copy(o[:, :nw], psm[:, :nw])
                    nc.sync.dma_start(
                        down[st * 128:(st + 1) * 128, n0:n0 + nw], o[:, :nw])
    nc.compile()
    return nc


def _build_phase2():
    bass, mybir, tile, bacc, _ = _bass_mods()
    f32 = mybir.dt.float32
    mdt = mybir.dt.bfloat16 if MM_DTYPE == "bf16" else mybir.dt.float32r
    Act = mybir.ActivationFunctionType
    nc = bacc.Bacc("TRN2", target_bir_lowering=False, debug=False,
                   num_devices=NCORES)
    qcT = nc.dram_tensor("qcT", (QLR, S), mdt, kind="ExternalInput").ap()
    ckvT = nc.dram_tensor("ckvT", (KVLR, S), mdt, kind="ExternalInput").ap()
    krT = nc.dram_tensor("krT", (2 * ROPE, S), mdt, kind="ExternalInput").ap()
    wquT = nc.dram_tensor("wquT", (QLR, 4 * 128), mdt, kind="ExternalInput").ap()
    wkkT = nc.dram_tensor("wkkT", (KVLR, HL * NOPE), mdt, kind="ExternalInput").ap()
    wkvT = nc.dram_tensor("wkvT", (KVLR, HL * VD), mdt, kind="ExternalInput").ap()
    woT = nc.dram_tensor("woT", (HL * VD, HID), mdt, kind="ExternalInput").ap()
    cosT = nc.dram_tensor("cosT", (2 * ROPE, S), f32, kind="ExternalInput").ap()
    sinT = nc.dram_tensor("sinT", (2 * ROPE, S), f32, kind="ExternalInput").ap()
    maskD = nc.dram_tensor("mask", (4, 128, SQ), mdt, kind="ExternalInput").ap()
    attnT = nc.dram_tensor("attnT", (HL, S, S), f32, kind="ExternalOutput").ap()
    outT = nc.dram_tensor("outT", (HID, S), f32, kind="ExternalOutput").ap()

    KQ = QLR // 128   # 12
    KC = KVLR // 128  # 4
    with tile.TileContext(nc) as tc:
        with (
            tc.tile_pool(name="pers", bufs=1) as pers,
            tc.tile_pool(name="psA", bufs=2, space="PSUM") as psA,
            tc.tile_pool(name="psS", bufs=3, space="PSUM") as psS,
            tc.tile_pool(name="psC", bufs=1, space="PSUM") as psC,
            tc.tile_pool(name="psD", bufs=1, space="PSUM") as psD,
            tc.tile_pool(name="psB", bufs=1, space="PSUM") as psB,
        ):
            # ---- persistent tiles ----
            qn = [pers.tile([128, S], mdt, name=f"qn{h}") for h in range(HL)]
            qr2 = pers.tile([128, S], mdt, name="qr2")
            kn = [pers.tile([128, S], mdt, name=f"kn{h}") for h in range(HL)]
            kr = pers.tile([128, S], mdt, name="kr")
            vt = [pers.tile([128, HL * VD], mdt, name=f"v{i}") for i in range(NKB)]
            ctxt = [pers.tile([128, S], mdt, name=f"ctxt{h}") for h in range(HL)]
            mask_t = pers.tile([128, 4 * SQ], mdt, name="mask_t")
            onesf = pers.tile([128, 1], f32, name="onesf")
            ones_c = pers.tile([128, 1], mdt, name="ones_c")
            onesrf = pers.tile([1, 128], f32, name="onesrf")
            ones_r = pers.tile([1, 128], mdt, name="ones_r")

            nc.vector.memset(onesf[:], 1.0)
            nc.vector.tensor_copy(ones_c[:], onesf[:])
            nc.vector.memset(onesrf[:], 1.0)
            nc.vector.tensor_copy(ones_r[:], onesrf[:])
            for i in range(4):
                nc.sync.dma_start(mask_t[:, i * SQ:(i + 1) * SQ], maskD[i])
            nc.sync.dma_start(kr[:, :], krT[:, :])
            wo = pers.tile([128, HL * HID], mdt, name="wo")
            for hh in range(HL):
                nc.sync.dma_start(wo[:, hh * HID:(hh + 1) * HID],
                                  woT[hh * 128:(hh + 1) * 128, :])

            # ---- stage A: projections ----
            with tc.tile_pool(name="stA", bufs=1) as sa:
                wqu = sa.tile([128, KQ * 512], mdt, name="wqu")
                for k in range(KQ):
                    nc.sync.dma_start(
                        wqu[:, k * 512:(k + 1) * 512],
                        wquT[k * 128:(k + 1) * 128, :])
                ckv = sa.tile([128, KC * S], mdt, name="ckv")
                for k in range(KC):
                    nc.sync.dma_start(ckv[:, k * S:(k + 1) * S],
                                      ckvT[k * 128:(k + 1) * 128, :])
                wkk = sa.tile([128, KC * HL * NOPE], mdt, name="wkk")
                wkv = sa.tile([128, KC * HL * VD], mdt, name="wkv")
                for k in range(KC):
                    nc.sync.dma_start(wkk[:, k * HL * NOPE:(k + 1) * HL * NOPE],
                                      wkkT[k * 128:(k + 1) * 128, :])
                    nc.sync.dma_start(wkv[:, k * HL * VD:(k + 1) * HL * VD],
                                      wkvT[k * 128:(k + 1) * 128, :])

                W = 4 * 128  # 512
                qcs = []
                for k in range(KQ):
                    q = sa.tile([128, S], mdt, name="qcs", tag="qcs", bufs=12)
                    nc.sync.dma_start(q[:], qcT[k * 128:(k + 1) * 128, :])
                    qcs.append(q)
                cst = sa.tile([128, S], f32, name="cst")
                snt = sa.tile([128, S], f32, name="snt")
                nc.sync.dma_start(cst[:], cosT[:, :])
                nc.sync.dma_start(snt[:], sinT[:, :])
                for si in range(NSI):
                    sl = slice(si * SQ, (si + 1) * SQ)
                    psms = []
                    for m in range(4):
                        psm = psA.tile([128, 512], f32, name="pA", tag="pA")
                        for k in range(KQ):
                            nc.tensor.matmul(
                                psm[:],
                                wqu[:, k * W + m * 128: k * W + (m + 1) * 128],
                                qcs[k][:, sl],
                                start=(k == 0), stop=(k == KQ - 1))
                        if m < HL:
                            nc.any.tensor_copy(qn[m][:, sl], psm[:])
                        else:
                            psms.append(psm)
                    t1 = sa.tile([128, SQ], f32, name="t1", tag="t1", bufs=3)
                    t2 = sa.tile([128, SQ], f32, name="t2", tag="t2", bufs=3)
                    nc.vector.tensor_mul(t1[:], psms[0][:], cst[:, sl])
                    nc.vector.tensor_mul(t2[:], psms[1][:], snt[:, sl])
                    nc.vector.tensor_add(qr2[:, sl], t1[:], t2[:])

                    for m in range(HL):
                        psm = psA.tile([128, 512], f32, name="pA", tag="pA")
                        for k in range(KC):
                            nc.tensor.matmul(
                                psm[:],
                                wkk[:, k * HL * NOPE + m * 128: k * HL * NOPE + (m + 1) * 128],
                                ckv[:, k * S + si * SQ: k * S + (si + 1) * SQ],
                                start=(k == 0), stop=(k == KC - 1))
                        nc.any.tensor_copy(kn[m][:, sl], psm[:])

                for sb16 in range(NKB):
                    psm = psA.tile([128, 512], f32, name="pA", tag="pA")
                    for k in range(KC):
                        nc.tensor.matmul(
                            psm[:, :HL * VD],
                            ckv[:, k * S + sb16 * 128: k * S + (sb16 + 1) * 128],
                            wkv[:, k * HL * VD:(k + 1) * HL * VD],
                            start=(k == 0), stop=(k == KC - 1))
                    nc.any.tensor_copy(vt[sb16][:], psm[:, :HL * VD])

            # ---- stage B: attention + fused out-projection ----
            with tc.tile_pool(name="stB", bufs=1) as sbp:
                for si in range(NSI):
                    sl = slice(si * SQ, (si + 1) * SQ)
                    nki = 4 * si + 4
                    for h in range(HL):
                        den = psD.tile([1, 512], f32, name="den", tag="den")
                        ctxp = psC.tile([128, 512], f32, name="ctxp", tag="ctxp")
                        exps = []
                        for ki in range(nki):
                            pss = psS.tile([128, 512], f32, name="pS", tag="pS")
                            nc.tensor.matmul(pss[:],
                                             kn[h][:, ki * 128:(ki + 1) * 128],
                                             qn[h][:, sl], start=True, stop=False)
                            nc.tensor.matmul(pss[:],
                                             kr[h * 64:(h + 1) * 64, ki * 128:(ki + 1) * 128],
                                             qr2[h * 64:(h + 1) * 64, sl],
                                             start=False, stop=True)
                            e = sbp.tile([128, SQ], mdt, name="exp", tag="exp", bufs=18)
                            nc.scalar.activation(e[:], pss[:], Act.Exp, scale=SCALE)
                            if ki >= 4 * si:
                                m = ki - 4 * si
                                nc.vector.tensor_mul(
                                    e[:], e[:], mask_t[:, m * SQ:(m + 1) * SQ])
                            nc.tensor.matmul(den[:], ones_c[:], e[:],
                                             start=(ki == 0), stop=(ki == nki - 1))
                            nc.tensor.matmul(ctxp[:], vt[ki][:, h * VD:(h + 1) * VD],
                                             e[:], start=(ki == 0), stop=(ki == nki - 1))
                            exps.append(e)
                        rec = sbp.tile([1, 512], f32, name="rec", tag="rec", bufs=2)
                        nc.vector.reciprocal(rec[:], den[:])
                        recr = sbp.tile([1, 512], mdt, name="recr", tag="recr", bufs=2)
                        nc.vector.tensor_copy(recr[:], rec[:])
                        bcp = psB.tile([128, 512], f32, name="bcp", tag="bcp")
                        nc.tensor.matmul(bcp[:], ones_r[:], recr[:], start=True, stop=True)
                        bc = sbp.tile([128, 512], f32, name="bc", tag="bc", bufs=2)
                        nc.any.tensor_copy(bc[:], bcp[:])
                        for ki in range(nki):
                            at = sbp.tile([128, SQ], f32, name="at", tag="at", bufs=4)
                            nc.vector.tensor_mul(at[:], exps[ki][:], bc[:])
                            nc.sync.dma_start(
                                attnT[h, ki * 128:(ki + 1) * 128, sl], at[:])
                        nc.vector.tensor_mul(ctxt[h][:, sl], ctxp[:], bc[:])

                    # out projection for this query tile (both heads ready)
                    for mi in range(HID // 128):
                        psm = psA.tile([128, 512], f32, name="pA", tag="pA")
                        for hh in range(HL):
                            nc.tensor.matmul(
                                psm[:],
                                wo[:, hh * HID + mi * 128: hh * HID + (mi + 1) * 128],
                                ctxt[hh][:, sl],
                                start=(hh == 0), stop=(hh == HL - 1))
                        o = sbp.tile([128, 512], f32, name="oout", tag="oout", bufs=4)
                        nc.any.tensor_copy(o[:], psm[:])
                        nc.sync.dma_start(outT[mi * 128:(mi + 1) * 128, sl], o[:])
    nc.compile()
    return nc


# ---------------- optional NTFF profiling (enabled via MLA_PROF_DIR) ---------
@contextlib.contextmanager
def _maybe_profile(tag):
    prof_dir = os.environ.get("MLA_PROF_DIR")
    if not prof_dir:
        yield
        return
    so = "/opt/axon/libaxon_pjrt.so"
    try:
        import jax
        jax.devices()
        lib = ctypes.CDLL(so)
        lib.axon_start_nrt_profile.argtypes = [ctypes.POINTER(ctypes.c_int64),
                                               ctypes.c_size_t]
        lib.axon_start_nrt_profile.restype = ctypes.c_int64
        lib.axon_stop_nrt_profile.argtypes = [ctypes.c_char_p]
        lib.axon_stop_nrt_profile.restype = ctypes.c_int64
        out = os.path.join(prof_dir, tag)
        os.makedirs(out, exist_ok=True)
        for f in _glob.glob(os.path.join(out, "*")):
            os.unlink(f)
        rc = lib.axon_start_nrt_profile(None, 0)
        if rc != 0:
            raise RuntimeError(f"start_nrt_profile rc={rc}")
        try:
            yield
        finally:
            n = lib.axon_stop_nrt_profile(out.encode())
            print(f"[prof] {tag}: {n} ntff files -> {out}", file=sys.stderr)
    except Exception as exc:  # profiling is best-effort only
        print(f"[prof] {tag} unavailable: {exc}", file=sys.stderr)
        yield


def _run(nc, in_maps, tag):
    _, _, _, _, bass_utils = _bass_mods()
    if os.environ.get("MLA_PROF_DIR"):
        # warm-up run (compile + first execute) outside the profile capture
        bass_utils.run_bass_kernel_spmd(nc, in_maps, core_ids=list(range(NCORES)))
    with _maybe_profile(tag):
        res = bass_utils.run_bass_kernel_spmd(
            nc, in_maps, core_ids=list(range(NCORES)))
    return res.results


# ---------------- host orchestration ----------------------------------------
def _rmsnorm(x, g):
    var = np.mean(x * x, axis=-1, keepdims=True, dtype=np.float32)
    return (x * (1.0 / np.sqrt(var + EPS))) * g


def kernel(**inputs):
    hidden = np.ascontiguousarray(np.asarray(inputs["hidden_states"],
                                             dtype=np.float32)[0])   # [S, HID]
    pos = np.asarray(inputs["position_ids"]).astype(np.int64)[0]     # [S]
    Wq_down = np.asarray(inputs["Wq_down"], dtype=np.float32)
    bq_down = np.asarray(inputs["bq_down"], dtype=np.float32)
    gq_norm = np.asarray(inputs["gq_norm"], dtype=np.float32)
    Wq_up = np.asarray(inputs["Wq_up"], dtype=np.float32)
    Wkv_down = np.asarray(inputs["Wkv_down"], dtype=np.float32)
    bkv_down = np.asarray(inputs["bkv_down"], dtype=np.float32)
    gkv_norm = np.asarray(inputs["gkv_norm"], dtype=np.float32)
    Wkv_up = np.asarray(inputs["Wkv_up"], dtype=np.float32)
    Wo = np.asarray(inputs["Wo"], dtype=np.float32)

    # ---- phase 1: fused down-projection, sequence-sharded ----
    if "p1" not in _built:
        _built["p1"] = _build_phase1()
    mdt_np = _np_mm_dtype()
    hT = np.ascontiguousarray(hidden.T).astype(mdt_np)               # [HID, S]
    wdT = np.ascontiguousarray(
        np.concatenate([Wq_down, Wkv_down], axis=0).T).astype(mdt_np)
    in1 = [{"hT": np.ascontiguousarray(hT[:, c * SC:(c + 1) * SC]), "wdT": wdT}
           for c in range(NCORES)]
    res1 = _run(_built["p1"], in1, "phase1")
    down = np.concatenate([res1[c]["down"] for c in range(NCORES)], axis=0)
    down = down + np.concatenate([bq_down, bkv_down])[None, :]

    q_c = _rmsnorm(down[:, :QLR], gq_norm)                           # [S, QLR]
    ckv = _rmsnorm(down[:, QLR:QLR + KVLR], gkv_norm)                # [S, KVLR]
    k_rope = down[:, QLR + KVLR:]                                    # [S, ROPE]

    # ---- RoPE tables / host k-RoPE ----
    perm = np.concatenate([np.arange(0, ROPE, 2), np.arange(1, ROPE, 2)])
    inv_freq = (1.0 / (THETA ** (np.arange(0, ROPE, 2, dtype=np.float32) / ROPE)))
    t = np.arange(S, dtype=np.float32)
    freqs = t[:, None] * inv_freq[None, :]
    emb = np.concatenate([freqs, freqs], axis=-1)                    # [S, ROPE]
    cos = np.cos(emb)[pos].astype(np.float32)                        # [S, ROPE]
    sin = np.sin(emb)[pos].astype(np.float32)
    kp = k_rope[:, perm]
    k_rot = np.empty_like(kp)
    half = ROPE // 2
    k_rot[:, :half] = kp[:, :half] * cos[:, :half] - kp[:, half:] * sin[:, :half]
    k_rot[:, half:] = kp[:, half:] * cos[:, half:] + kp[:, :half] * sin[:, half:]

    # ---- phase 2 inputs ----
    if "p2" not in _built:
        _built["p2"] = _build_phase2()
    qcT = np.ascontiguousarray(q_c.T).astype(mdt_np)                 # [QLR, S]
    ckvT = np.ascontiguousarray(ckv.T).astype(mdt_np)                # [KVLR, S]
    krT = np.ascontiguousarray(np.vstack([k_rot.T, k_rot.T])).astype(mdt_np)
    cosT2 = np.ascontiguousarray(np.vstack([cos.T, cos.T]))          # [128, S]
    sinT2 = np.ascontiguousarray(np.vstack([sin.T, sin.T]))
    mask = np.zeros((4, 128, SQ), dtype=np.float32)
    kk = np.arange(128)[:, None]
    ss = np.arange(SQ)[None, :]
    for m in range(4):
        mask[m] = (128 * m + kk <= ss).astype(np.float32)
    mask = mask.astype(mdt_np)

    in2 = []
    for c in range(NCORES):
        hs = [2 * c, 2 * c + 1]
        wq_cols = []
        for hh in hs:
            wq_cols.append(Wq_up[hh * QD:(hh + 1) * QD][:NOPE])
        for hh in hs:
            wq_cols.append(Wq_up[hh * QD:(hh + 1) * QD][NOPE:][perm])
        for hh in hs:
            wp = Wq_up[hh * QD:(hh + 1) * QD][NOPE:][perm]
            wq_cols.append(np.concatenate([-wp[32:], wp[:32]], axis=0))
        wquT = np.ascontiguousarray(np.concatenate(wq_cols, axis=0).T).astype(mdt_np)
        wkkT = np.ascontiguousarray(np.concatenate(
            [Wkv_up[hh * (NOPE + VD): hh * (NOPE + VD) + NOPE] for hh in hs],
            axis=0).T).astype(mdt_np)
        wkvT = np.ascontiguousarray(np.concatenate(
            [Wkv_up[hh * (NOPE + VD) + NOPE: (hh + 1) * (NOPE + VD)] for hh in hs],
            axis=0).T).astype(mdt_np)
        woT = np.ascontiguousarray(
            np.concatenate([Wo[:, hh * VD:(hh + 1) * VD] for hh in hs],
                           axis=1).T).astype(mdt_np)
        in2.append({"qcT": qcT, "ckvT": ckvT, "krT": krT,
                    "wquT": wquT, "wkkT": wkkT, "wkvT": wkvT, "woT": woT,
                    "cosT": cosT2, "sinT": sinT2, "mask": mask})
    res2 = _run(_built["p2"], in2, "phase2")

    # ---- host gather ----
    out_eT = np.zeros((HID, S), dtype=np.float32)
    for c in range(NCORES):
        out_eT += res2[c]["outT"]
    out = np.ascontiguousarray(out_eT.T)[None]                       # [1, S, HID]

    attn = np.empty((1, H, S, S), dtype=np.float32)
    for c in range(NCORES):
        aT = res2[c]["attnT"]                                        # [2, S(keys), S(q)]
        for hh in range(HL):
            attn[0, 2 * c + hh] = aT[hh].T
    return out, attn


# revision 9
# speedup vs baseline: 1.0636x; 1.0636x over previous
"""DeepSeek-V2 MLA attention on 8 Trainium2 NeuronCores (Bass/Tile).

Strategy:
  Phase 1 (sequence-sharded): each core computes its 256-row slice of the
    fused down-projection  hidden @ [Wq_down | Wkv_down].T.
  Host glue: bias add, RMSNorm, k-RoPE, transposes (O(S*D) only).
  Phase 2 (head-sharded, 2 heads/core): q/kv up-projections, on-device
    q-RoPE (rotate-half folded into an extra weight block), causal
    attention with scores held transposed [keys, queries], softmax
    denominator via ones-matmul, ctx, and a fused partial out-projection
    interleaved per query tile.
  Host gather: sum the 8 partial out projections, transpose attn.

Matmul dtype: bf16 (fp32 PSUM accumulation) by default; MLA_DTYPE=f32r
switches to full-rate fp32r.
"""
import contextlib
import ctypes
import glob as _glob
import os
import sys

import numpy as np

# The grading environment may pin JAX_PLATFORMS=cpu (needed for the jax
# reference); the axon trn2 devices must stay visible for this kernel.
if os.environ.get("JAX_PLATFORMS", "").lower() in ("cpu",):
    os.environ["JAX_PLATFORMS"] = ""

B, S, HID = 1, 2048, 2048
H = 16
NOPE, ROPE, VD = 128, 64, 128
QD = NOPE + ROPE
QLR, KVLR = 1536, 512
THETA = 10000.0
EPS = 1e-6

NCORES = 8
HL = H // NCORES            # heads per core
SC = S // NCORES            # phase-1 rows per core
DOWN = QLR + KVLR + ROPE    # 2112
SQ = 512                    # query-tile (matmul free dim)
NSI = S // SQ               # 4
NKB = S // 128              # 16 key blocks
SCALE = 1.0 / float(np.sqrt(QD))

MM_DTYPE = os.environ.get("MLA_DTYPE", "bf16")   # "bf16" | "f32r"

_built = {}


def _np_mm_dtype():
    if MM_DTYPE == "bf16":
        import ml_dtypes
        return ml_dtypes.bfloat16
    return np.float32


def _bass_mods():
    import concourse.bass as bass
    import concourse.mybir as mybir
    import concourse.tile as tile
    from concourse import bacc, bass_utils
    return bass, mybir, tile, bacc, bass_utils


def _build_phase1():
    bass, mybir, tile, bacc, _ = _bass_mods()
    f32 = mybir.dt.float32
    mdt = mybir.dt.bfloat16 if MM_DTYPE == "bf16" else mybir.dt.float32r
    nc = bacc.Bacc("TRN2", target_bir_lowering=False, debug=False,
                   num_devices=NCORES)
    hT = nc.dram_tensor("hT", (HID, SC), mdt, kind="ExternalInput").ap()
    wdT = nc.dram_tensor("wdT", (HID, DOWN), mdt, kind="ExternalInput").ap()
    down = nc.dram_tensor("down", (SC, DOWN), f32, kind="ExternalOutput").ap()

    NK = HID // 128
    ntiles = [(0, 512), (512, 512), (1024, 512), (1536, 512), (2048, 64)]
    with tile.TileContext(nc) as tc:
        with (
            tc.tile_pool(name="sb", bufs=1) as sb,
            tc.tile_pool(name="out", bufs=2) as ob,
            tc.tile_pool(name="ps", bufs=1, space="PSUM") as ps,
        ):
            hT_t = sb.tile([128, NK * SC], mdt, name="hT_t")
            wd_t = sb.tile([128, NK * DOWN], mdt, name="wd_t")
            for k in range(NK):
                nc.sync.dma_start(hT_t[:, k * SC:(k + 1) * SC],
                                  hT[k * 128:(k + 1) * 128, :])
                nc.sync.dma_start(wd_t[:, k * DOWN:(k + 1) * DOWN],
                                  wdT[k * 128:(k + 1) * 128, :])
            # k-inner loops consume weight chunks as they stream in; psums
            # for all n-tiles of one row block accumulate concurrently.
            for st in range(SC // 128):
                psms = [ps.tile([128, nw], f32, name=f"psm{j}", tag=f"p{j}")
                        for j, (n0, nw) in enumerate(ntiles)]
                for k in range(NK):
                    for j, (n0, nw) in enumerate(ntiles):
                        nc.tensor.matmul(
                            psms[j][:],
                            hT_t[:, k * SC + st * 128: k * SC + (st + 1) * 128],
                            wd_t[:, k * DOWN + n0: k * DOWN + n0 + nw],
                            start=(k == 0), stop=(k == NK - 1))
                for j, (n0, nw) in enumerate(ntiles):
                    o = ob.tile([128, nw], f32, name=f"o{j}", tag=f"o{j}")
                    nc.any.tensor_copy(o[:], psms[j][:])
                    nc.sync.dma_start(
                        down[st * 128:(st + 1) * 128, n0:n0 + nw], o[:])
    nc.compile()
    return nc


def _build_phase2():
    bass, mybir, tile, bacc, _ = _bass_mods()
    f32 = mybir.dt.float32
    mdt = mybir.dt.bfloat16 if MM_DTYPE == "bf16" else mybir.dt.float32r
    Act = mybir.ActivationFunctionType
    nc = bacc.Bacc("TRN2", target_bir_lowering=False, debug=False,
                   num_devices=NCORES)
    qcT = nc.dram_tensor("qcT", (QLR, S), mdt, kind="ExternalInput").ap()
    ckvT = nc.dram_tensor("ckvT", (KVLR, S), mdt, kind="ExternalInput").ap()
    krT = nc.dram_tensor("krT", (2 * ROPE, S), mdt, kind="ExternalInput").ap()
    wquT = nc.dram_tensor("wquT", (QLR, 4 * 128), mdt, kind="ExternalInput").ap()
    wkkT = nc.dram_tensor("wkkT", (KVLR, HL * NOPE), mdt, kind="ExternalInput").ap()
    wkvT = nc.dram_tensor("wkvT", (KVLR, HL * VD), mdt, kind="ExternalInput").ap()
    woT = nc.dram_tensor("woT", (HL * VD, HID), mdt, kind="ExternalInput").ap()
    cosT = nc.dram_tensor("cosT", (2 * ROPE, S), f32, kind="ExternalInput").ap()
    sinT = nc.dram_tensor("sinT", (2 * ROPE, S), f32, kind="ExternalInput").ap()
    maskD = nc.dram_tensor("mask", (4, 128, SQ), mdt, kind="ExternalInput").ap()
    attnT = nc.dram_tensor("attnT", (HL, S, S), mdt, kind="ExternalOutput").ap()
    outT = nc.dram_tensor("outT", (HID, S), f32, kind="ExternalOutput").ap()

    KQ = QLR // 128   # 12
    KC = KVLR // 128  # 4
    with tile.TileContext(nc) as tc:
        with (
            tc.tile_pool(name="pers", bufs=1) as pers,
            tc.tile_pool(name="psA", bufs=2, space="PSUM") as psA,
            tc.tile_pool(name="psS", bufs=2, space="PSUM") as psS,
            tc.tile_pool(name="psC", bufs=2, space="PSUM") as psC,
            tc.tile_pool(name="psD", bufs=2, space="PSUM") as psD,
        ):
            # ---- persistent tiles ----
            qn = [pers.tile([128, S], mdt, name=f"qn{h}") for h in range(HL)]
            qr2 = pers.tile([128, S], mdt, name="qr2")
            kn = [pers.tile([128, S], mdt, name=f"kn{h}") for h in range(HL)]
            kr = pers.tile([128, S], mdt, name="kr")
            vt = [pers.tile([128, HL * VD], mdt, name=f"v{i}") for i in range(NKB)]
            ctxt = [pers.tile([128, S], mdt, name=f"ctxt{h}") for h in range(HL)]
            mask_t = pers.tile([128, 4 * SQ], mdt, name="mask_t")
            wo = pers.tile([128, HL * HID], mdt, name="wo")
            onesf = pers.tile([128, 1], f32, name="onesf")
            ones_c = pers.tile([128, 1], mdt, name="ones_c")

            nc.vector.memset(onesf[:], 1.0)
            nc.vector.tensor_copy(ones_c[:], onesf[:])

            # ---- stage A: projections ----
            with tc.tile_pool(name="stA", bufs=1) as sa:
                # DMAs ordered so the q-up pipeline can start immediately.
                wqu = sa.tile([128, KQ * 512], mdt, name="wqu")
                for k in range(KQ):
                    nc.sync.dma_start(
                        wqu[:, k * 512:(k + 1) * 512],
                        wquT[k * 128:(k + 1) * 128, :])
                qcs = []
                for k in range(KQ):
                    q = sa.tile([128, S], mdt, name="qcs", tag="qcs", bufs=12)
                    nc.sync.dma_start(q[:], qcT[k * 128:(k + 1) * 128, :])
                    qcs.append(q)
                cst = sa.tile([128, S], f32, name="cst")
                snt = sa.tile([128, S], f32, name="snt")
                nc.sync.dma_start(cst[:], cosT[:, :])
                nc.sync.dma_start(snt[:], sinT[:, :])
                ckv = sa.tile([128, KC * S], mdt, name="ckv")
                for k in range(KC):
                    nc.sync.dma_start(ckv[:, k * S:(k + 1) * S],
                                      ckvT[k * 128:(k + 1) * 128, :])
                wkk = sa.tile([128, KC * HL * NOPE], mdt, name="wkk")
                wkv = sa.tile([128, KC * HL * VD], mdt, name="wkv")
                for k in range(KC):
                    nc.sync.dma_start(wkk[:, k * HL * NOPE:(k + 1) * HL * NOPE],
                                      wkkT[k * 128:(k + 1) * 128, :])
                    nc.sync.dma_start(wkv[:, k * HL * VD:(k + 1) * HL * VD],
                                      wkvT[k * 128:(k + 1) * 128, :])
                # stage-B-only inputs, queued last
                for i in range(4):
                    nc.sync.dma_start(mask_t[:, i * SQ:(i + 1) * SQ], maskD[i])
                nc.sync.dma_start(kr[:, :], krT[:, :])
                for hh in range(HL):
                    nc.sync.dma_start(wo[:, hh * HID:(hh + 1) * HID],
                                      woT[hh * 128:(hh + 1) * 128, :])

                W = 4 * 128  # 512
                for si in range(NSI):
                    sl = slice(si * SQ, (si + 1) * SQ)
                    psms = []
                    for m in range(4):
                        psm = psA.tile([128, 512], f32, name="pA", tag="pA")
                        for k in range(KQ):
                            nc.tensor.matmul(
                                psm[:],
                                wqu[:, k * W + m * 128: k * W + (m + 1) * 128],
                                qcs[k][:, sl],
                                start=(k == 0), stop=(k == KQ - 1))
                        if m < HL:
                            nc.any.tensor_copy(qn[m][:, sl], psm[:])
                        else:
                            psms.append(psm)
                    t1 = sa.tile([128, SQ], f32, name="t1", tag="t1", bufs=3)
                    t2 = sa.tile([128, SQ], f32, name="t2", tag="t2", bufs=3)
                    nc.vector.tensor_mul(t1[:], psms[0][:], cst[:, sl])
                    nc.vector.tensor_mul(t2[:], psms[1][:], snt[:, sl])
                    nc.vector.tensor_add(qr2[:, sl], t1[:], t2[:])

                    for m in range(HL):
                        psm = psA.tile([128, 512], f32, name="pA", tag="pA")
                        for k in range(KC):
                            nc.tensor.matmul(
                                psm[:],
                                wkk[:, k * HL * NOPE + m * 128: k * HL * NOPE + (m + 1) * 128],
                                ckv[:, k * S + si * SQ: k * S + (si + 1) * SQ],
                                start=(k == 0), stop=(k == KC - 1))
                        nc.any.tensor_copy(kn[m][:, sl], psm[:])

                for sb16 in range(NKB):
                    psm = psA.tile([128, 512], f32, name="pA", tag="pA")
                    for k in range(KC):
                        nc.tensor.matmul(
                            psm[:, :HL * VD],
                            ckv[:, k * S + sb16 * 128: k * S + (sb16 + 1) * 128],
                            wkv[:, k * HL * VD:(k + 1) * HL * VD],
                            start=(k == 0), stop=(k == KC - 1))
                    nc.any.tensor_copy(vt[sb16][:], psm[:, :HL * VD])

            # ---- stage B: attention + fused out-projection ----
            with tc.tile_pool(name="stB", bufs=1) as sbp:
                for si in range(NSI):
                    sl = slice(si * SQ, (si + 1) * SQ)
                    nki = 4 * si + 4
                    for h in range(HL):
                        den = psD.tile([1, 512], f32, name="den", tag="den")
                        ctxp = psC.tile([128, 512], f32, name="ctxp", tag="ctxp")
                        exps = []
                        for ki in range(nki):
                            pss = psS.tile([128, 512], f32, name="pS", tag="pS")
                            nc.tensor.matmul(pss[:],
                                             kn[h][:, ki * 128:(ki + 1) * 128],
                                             qn[h][:, sl], start=True, stop=False)
                            nc.tensor.matmul(pss[:],
                                             kr[h * 64:(h + 1) * 64, ki * 128:(ki + 1) * 128],
                                             qr2[h * 64:(h + 1) * 64, sl],
                                             start=False, stop=True)
                            e = sbp.tile([128, SQ], mdt, name="exp", tag="exp", bufs=18)
                            nc.scalar.activation(e[:], pss[:], Act.Exp, scale=SCALE)
                            if ki >= 4 * si:
                                m = ki - 4 * si
                                nc.vector.tensor_mul(
                                    e[:], e[:], mask_t[:, m * SQ:(m + 1) * SQ])
                            nc.tensor.matmul(den[:], ones_c[:], e[:],
                                             start=(ki == 0), stop=(ki == nki - 1))
                            nc.tensor.matmul(ctxp[:], vt[ki][:, h * VD:(h + 1) * VD],
                                             e[:], start=(ki == 0), stop=(ki == nki - 1))
                            exps.append(e)
                        rec = sbp.tile([1, 512], f32, name="rec", tag="rec", bufs=2)
                        nc.vector.reciprocal(rec[:], den[:])
                        bc = sbp.tile([128, 512], f32, name="bc", tag="bc", bufs=2)
                        nc.gpsimd.partition_broadcast(bc[:], rec[:])
                        for ki in range(nki):
                            at = sbp.tile([128, SQ], mdt, name="at", tag="at", bufs=4)
                            nc.vector.tensor_mul(at[:], exps[ki][:], bc[:])
                            nc.sync.dma_start(
                                attnT[h, ki * 128:(ki + 1) * 128, sl], at[:])
                        nc.vector.tensor_mul(ctxt[h][:, sl], ctxp[:], bc[:])

                    # out projection for this query tile (both heads ready)
                    for mi in range(HID // 128):
                        psm = psA.tile([128, 512], f32, name="pA", tag="pA")
                        for hh in range(HL):
                            nc.tensor.matmul(
                                psm[:],
                                wo[:, hh * HID + mi * 128: hh * HID + (mi + 1) * 128],
                                ctxt[hh][:, sl],
                                start=(hh == 0), stop=(hh == HL - 1))
                        o = sbp.tile([128, 512], f32, name="oout", tag="oout", bufs=4)
                        nc.any.tensor_copy(o[:], psm[:])
                        nc.sync.dma_start(outT[mi * 128:(mi + 1) * 128, sl], o[:])
    nc.compile()
    return nc


# ---------------- optional NTFF profiling (enabled via MLA_PROF_DIR) ---------
@contextlib.contextmanager
def _maybe_profile(tag):
    prof_dir = os.environ.get("MLA_PROF_DIR")
    if not prof_dir:
        yield
        return
    so = "/opt/axon/libaxon_pjrt.so"
    try:
        import jax
        jax.devices()
        lib = ctypes.CDLL(so)
        lib.axon_start_nrt_profile.argtypes = [ctypes.POINTER(ctypes.c_int64),
                                               ctypes.c_size_t]
        lib.axon_start_nrt_profile.restype = ctypes.c_int64
        lib.axon_stop_nrt_profile.argtypes = [ctypes.c_char_p]
        lib.axon_stop_nrt_profile.restype = ctypes.c_int64
        out = os.path.join(prof_dir, tag)
        os.makedirs(out, exist_ok=True)
        for f in _glob.glob(os.path.join(out, "*")):
            os.unlink(f)
        rc = lib.axon_start_nrt_profile(None, 0)
        if rc != 0:
            raise RuntimeError(f"start_nrt_profile rc={rc}")
        try:
            yield
        finally:
            n = lib.axon_stop_nrt_profile(out.encode())
            print(f"[prof] {tag}: {n} ntff files -> {out}", file=sys.stderr)
    except Exception as exc:  # profiling is best-effort only
        print(f"[prof] {tag} unavailable: {exc}", file=sys.stderr)
        yield


def _run(nc, in_maps, tag):
    _, _, _, _, bass_utils = _bass_mods()
    if os.environ.get("MLA_PROF_DIR"):
        # warm-up run (compile + first execute) outside the profile capture
        bass_utils.run_bass_kernel_spmd(nc, in_maps, core_ids=list(range(NCORES)))
    with _maybe_profile(tag):
        res = bass_utils.run_bass_kernel_spmd(
            nc, in_maps, core_ids=list(range(NCORES)))
    return res.results


# ---------------- host orchestration ----------------------------------------
def _rmsnorm(x, g):
    var = np.mean(x * x, axis=-1, keepdims=True, dtype=np.float32)
    return (x * (1.0 / np.sqrt(var + EPS))) * g


def kernel(**inputs):
    hidden = np.ascontiguousarray(np.asarray(inputs["hidden_states"],
                                             dtype=np.float32)[0])   # [S, HID]
    pos = np.asarray(inputs["position_ids"]).astype(np.int64)[0]     # [S]
    Wq_down = np.asarray(inputs["Wq_down"], dtype=np.float32)
    bq_down = np.asarray(inputs["bq_down"], dtype=np.float32)
    gq_norm = np.asarray(inputs["gq_norm"], dtype=np.float32)
    Wq_up = np.asarray(inputs["Wq_up"], dtype=np.float32)
    Wkv_down = np.asarray(inputs["Wkv_down"], dtype=np.float32)
    bkv_down = np.asarray(inputs["bkv_down"], dtype=np.float32)
    gkv_norm = np.asarray(inputs["gkv_norm"], dtype=np.float32)
    Wkv_up = np.asarray(inputs["Wkv_up"], dtype=np.float32)
    Wo = np.asarray(inputs["Wo"], dtype=np.float32)

    # ---- phase 1: fused down-projection, sequence-sharded ----
    if "p1" not in _built:
        _built["p1"] = _build_phase1()
    mdt_np = _np_mm_dtype()
    hT = np.ascontiguousarray(hidden.T).astype(mdt_np)               # [HID, S]
    wdT = np.ascontiguousarray(
        np.concatenate([Wq_down, Wkv_down], axis=0).T).astype(mdt_np)
    in1 = [{"hT": np.ascontiguousarray(hT[:, c * SC:(c + 1) * SC]), "wdT": wdT}
           for c in range(NCORES)]
    res1 = _run(_built["p1"], in1, "phase1")
    down = np.concatenate([res1[c]["down"] for c in range(NCORES)], axis=0)
    down = down + np.concatenate([bq_down, bkv_down])[None, :]

    q_c = _rmsnorm(down[:, :QLR], gq_norm)                           # [S, QLR]
    ckv = _rmsnorm(down[:, QLR:QLR + KVLR], gkv_norm)                # [S, KVLR]
    k_rope = down[:, QLR + KVLR:]                                    # [S, ROPE]

    # ---- RoPE tables / host k-RoPE ----
    perm = np.concatenate([np.arange(0, ROPE, 2), np.arange(1, ROPE, 2)])
    inv_freq = (1.0 / (THETA ** (np.arange(0, ROPE, 2, dtype=np.float32) / ROPE)))
    t = np.arange(S, dtype=np.float32)
    freqs = t[:, None] * inv_freq[None, :]
    emb = np.concatenate([freqs, freqs], axis=-1)                    # [S, ROPE]
    cos = np.cos(emb)[pos].astype(np.float32)                        # [S, ROPE]
    sin = np.sin(emb)[pos].astype(np.float32)
    kp = k_rope[:, perm]
    k_rot = np.empty_like(kp)
    half = ROPE // 2
    k_rot[:, :half] = kp[:, :half] * cos[:, :half] - kp[:, half:] * sin[:, :half]
    k_rot[:, half:] = kp[:, half:] * cos[:, half:] + kp[:, :half] * sin[:, half:]

    # ---- phase 2 inputs ----
    if "p2" not in _built:
        _built["p2"] = _build_phase2()
    qcT = np.ascontiguousarray(q_c.T).astype(mdt_np)                 # [QLR, S]
    ckvT = np.ascontiguousarray(ckv.T).astype(mdt_np)                # [KVLR, S]
    krT = np.ascontiguousarray(np.vstack([k_rot.T, k_rot.T])).astype(mdt_np)
    cosT2 = np.ascontiguousarray(np.vstack([cos.T, cos.T]))          # [128, S]
    sinT2 = np.ascontiguousarray(np.vstack([sin.T, sin.T]))
    mask = np.zeros((4, 128, SQ), dtype=np.float32)
    kk = np.arange(128)[:, None]
    ss = np.arange(SQ)[None, :]
    for m in range(4):
        mask[m] = (128 * m + kk <= ss).astype(np.float32)
    mask = mask.astype(mdt_np)

    in2 = []
    for c in range(NCORES):
        hs = [2 * c, 2 * c + 1]
        wq_cols = []
        for hh in hs:
            wq_cols.append(Wq_up[hh * QD:(hh + 1) * QD][:NOPE])
        for hh in hs:
            wq_cols.append(Wq_up[hh * QD:(hh + 1) * QD][NOPE:][perm])
        for hh in hs:
            wp = Wq_up[hh * QD:(hh + 1) * QD][NOPE:][perm]
            wq_cols.append(np.concatenate([-wp[32:], wp[:32]], axis=0))
        wquT = np.ascontiguousarray(np.concatenate(wq_cols, axis=0).T).astype(mdt_np)
        wkkT = np.ascontiguousarray(np.concatenate(
            [Wkv_up[hh * (NOPE + VD): hh * (NOPE + VD) + NOPE] for hh in hs],
            axis=0).T).astype(mdt_np)
        wkvT = np.ascontiguousarray(np.concatenate(
            [Wkv_up[hh * (NOPE + VD) + NOPE: (hh + 1) * (NOPE + VD)] for hh in hs],
            axis=0).T).astype(mdt_np)
        woT = np.ascontiguousarray(
            np.concatenate([Wo[:, hh * VD:(hh + 1) * VD] for hh in hs],
                           axis=1).T).astype(mdt_np)
        in2.append({"qcT": qcT, "ckvT": ckvT, "krT": krT,
                    "wquT": wquT, "wkkT": wkkT, "wkvT": wkvT, "woT": woT,
                    "cosT": cosT2, "sinT": sinT2, "mask": mask})
    res2 = _run(_built["p2"], in2, "phase2")

    # ---- host gather ----
    out_eT = np.zeros((HID, S), dtype=np.float32)
    for c in range(NCORES):
        out_eT += res2[c]["outT"]
    out = np.ascontiguousarray(out_eT.T)[None]                       # [1, S, HID]

    attn = np.empty((1, H, S, S), dtype=np.float32)
    for c in range(NCORES):
        aT = res2[c]["attnT"]                                        # [2, S(keys), S(q)]
        for hh in range(HL):
            attn[0, 2 * c + hh] = aT[hh].T.astype(np.float32)
    return out, attn
